# revision 57
# baseline (speedup 1.0000x reference)
"""Multi-head causal self-attention (B=2, T=4096, C=768, H=12, D=64) on 8 trn2 cores.

Sharding: core c -> batch b = c//4, head group g = c%4 (3 heads each).
Each core computes the qkv projection for its heads, causal attention, and a
row-parallel partial of the output projection; the host sums the 4 partials
per batch and adds b_out.

Device algorithm (per core), all matmuls bf16 with fp32 PSUM accumulation:
  qT/kT[h]  [64, T]  = (w_qk_h | b)^T @ (x | 1)^T          (bias via augmented row)
  v chunks  vsb [128, NKV, HPC, 65] with ones column (softmax denominators)
  Attention per group (head h, 512-wide query tile n): kv chunks j <= 4n+3 in
  PAIRS packed into one [128,1024] PSUM tile so a single ACT exp covers both
  (ACT is the bottleneck engine). Diagonal chunks are causally trimmed: chunk
  v=j-4n only computes q-cols >= 128v, packed tightly at staggered bases so
  the exp region stays contiguous. AV uses lhsT=P^T-blocks:
  acc[q 128, 65] += P^T-block^T @ v_aug (N=65); the 65th column accumulates
  the softmax denominator. PSUM accumulation groups must be SEQUENTIAL per
  bank (hardware restriction, verified empirically), so AV runs block-major
  (per q-block over all its chunks) and is deferred into the next group's
  pair loop through a thunk queue — which also drip-feeds projection and
  output-projection work between pairs to keep every engine busy.
  Normalization is fused into the PSUM->SBUF copy (tensor_scalar_mul by
  1/denominator per partition). a^T is produced by batched DMA XBAR
  transposes of [128q, 4, h0 d|h1 d] tiles landing directly in the stacked
  layout aT2 [128=h0|h1, T] / aT1 [64=h2, T]; the out-projection contracts
  K=128+64: y[q 128, 768] partial (bf16) -> DRAM.
"""

import sys

sys.path.insert(0, "/opt/trn_rl_repo")

from contextlib import ExitStack

import numpy as np

import concourse.bass as bass
import concourse.bacc as bacc
import concourse.mybir as mybir
from concourse import tile
from concourse.bass_utils import run_bass_kernel_spmd

B, T, C, H, D = 2, 4096, 768, 12, 64
HPC = 3  # heads per core
NCORES = 8
P = 128
NKV = T // P  # 32 kv chunks of 128
NI = T // 512  # 8 query super-tiles of 512
KC = C // P  # 6 full contraction chunks (+1 bias row)

BF16 = mybir.dt.bfloat16
F32 = mybir.dt.float32
NPBF16 = np.dtype(mybir.dt.np(BF16))

TRACE = False
EAGER_TAIL = True
DEBUG = False  # add intermediate DRAM dumps (dbg.py)
LAST = None  # last BassKernelResults

_prog = None
_last_in_maps = None


def bench(n=5):
    """Re-run the compiled kernel n times; returns per-run wall seconds."""
    import time

    times = []
    for _ in range(n):
        t0 = time.time()
        run_bass_kernel_spmd(_prog, _last_in_maps, list(range(NCORES)))
        times.append(time.time() - t0)
    return times


def _build():
    nc = bacc.Bacc(
        "TRN2",
        target_bir_lowering=False,
        debug=False,
        enable_asserts=False,
        num_devices=NCORES,
    )
    xt = nc.declare_dram_parameter("xt", [C + 1, T], BF16, False)
    wqk = nc.declare_dram_parameter("wqk", [C + 1, 2 * D * HPC], BF16, False)
    wv = nc.declare_dram_parameter("wv", [C + 1, D * HPC], BF16, False)
    wo2 = nc.declare_dram_parameter("wo2", [P, C], BF16, False)  # heads 0,1 rows
    wo1 = nc.declare_dram_parameter("wo1", [D, C], BF16, False)  # head 2 rows
    tri = nc.declare_dram_parameter("tri", [P, 2 * P], BF16, False)
    y = nc.declare_dram_parameter("y", [NI, 4, P, C], BF16, True)
    dbg = {}
    if DEBUG:
        for h in range(HPC):
            dbg[f"qT{h}_d"] = nc.declare_dram_parameter(f"qT{h}_d", [D, T], BF16, True)
            dbg[f"kT{h}_d"] = nc.declare_dram_parameter(f"kT{h}_d", [D, T], BF16, True)
        dbg["vsb_d"] = nc.declare_dram_parameter("vsb_d", [P, NKV, HPC, D + 1], BF16, True)
        dbg["aT2_d"] = nc.declare_dram_parameter("aT2_d", [P, T], BF16, True)
        dbg["aT1_d"] = nc.declare_dram_parameter("aT1_d", [P, T], BF16, True)

    with ExitStack() as ctx:
        tc = ctx.enter_context(tile.TileContext(nc))
        cp = ctx.enter_context(tc.tile_pool(name="const", bufs=1))
        ppt = ctx.enter_context(tc.tile_pool(name="ppt", bufs=22))
        pasb = ctx.enter_context(tc.tile_pool(name="pasb", bufs=2))
        pys = ctx.enter_context(tc.tile_pool(name="pys", bufs=2))
        prr = ctx.enter_context(tc.tile_pool(name="prr", bufs=2))
        psS = ctx.enter_context(tc.tile_pool(name="psS", bufs=2, space="PSUM"))
        psA = ctx.enter_context(tc.tile_pool(name="psA", bufs=1, space="PSUM"))
        psG = ctx.enter_context(tc.tile_pool(name="psG", bufs=3, space="PSUM"))

        # ---- constant tiles ----
        xtc = cp.tile([P, KC, T], BF16, tag="xtc", name="xtc")
        xtb = cp.tile([1, T], BF16, tag="xtb", name="xtb")
        wqkc = cp.tile([P, KC, 2 * D * HPC], BF16, tag="wqkc", name="wqkc")
        wqkb = cp.tile([1, 2 * D * HPC], BF16, tag="wqkb", name="wqkb")
        wvc = cp.tile([P, KC, D * HPC], BF16, tag="wvc", name="wvc")
        wvb = cp.tile([1, D * HPC], BF16, tag="wvb", name="wvb")
        xt_sb = [xtc[:, p, :] for p in range(KC)] + [xtb[:]]
        wqk_sb = [wqkc[:, p, :] for p in range(KC)] + [wqkb[:]]
        wv_sb = [wvc[:, p, :] for p in range(KC)] + [wvb[:]]
        wo2_sb = cp.tile([P, C], BF16, tag="wo2", name="wo2_sb")
        wo1_sb = cp.tile([D, C], BF16, tag="wo1", name="wo1_sb")
        tri_sb = cp.tile([P, 2 * P], BF16, tag="tri", name="tri_sb")
        qT = [cp.tile([D, T], BF16, tag=f"qT{h}", name=f"qT{h}") for h in range(HPC)]
        kT = [cp.tile([D, T], BF16, tag=f"kT{h}", name=f"kT{h}") for h in range(HPC)]
        vsb = cp.tile([P, NKV, HPC, D + 1], BF16, tag="v", name="vsb")
        aT2 = cp.tile([P, T], BF16, tag="aT2", name="aT2")  # rows 0:64 h0, 64:128 h1
        aT1 = cp.tile([P, T], BF16, tag="aT1", name="aT1")  # rows 0:64 h2

        # ---- input DMA, ordered so tile-0 projections can start ASAP ----
        xt_part = xt[0:C, :].rearrange("(c p) t -> p c t", p=P)
        sl0 = slice(0, 512)
        nc.sync.dma_start(xtc[:, :, sl0], xt_part[:, :, sl0])
        nc.sync.dma_start(wqkc[:], wqk[0:C, :].rearrange("(c p) w -> p c w", p=P))
        nc.sync.dma_start(xtb[:], xt[C : C + 1, :])
        nc.sync.dma_start(wqkb[:], wqk[C : C + 1, :])
        nc.sync.dma_start(wvc[:], wv[0:C, :].rearrange("(c p) w -> p c w", p=P))
        nc.sync.dma_start(wvb[:], wv[C : C + 1, :])
        nc.sync.dma_start(tri_sb[:], tri[:])
        nc.sync.dma_start(wo2_sb[:], wo2[:])
        nc.sync.dma_start(wo1_sb[:], wo1[:])
        nc.vector.memset(vsb[:, :, :, D : D + 1], 1.0)
        wup = cp.tile([P, P], BF16, tag="wup", name="wup")
        nc.vector.memset(wup[:], 0.0)
        wt = psA.tile([P, 4, D + 1], F32, tag="ac", name="warm_ps")
        for i in range(160):
            nc.tensor.matmul(
                wt[:, 0, 0:D], wup[:], wup[:, 0:D], start=(i == 0), stop=(i == 159)
            )
        for n in range(1, NI):
            sl = slice(512 * n, 512 * (n + 1))
            nc.sync.dma_start(xtc[:, :, sl], xt_part[:, :, sl])

        # ---- projection / epilogue emitters ----
        def qk_proj(h, n):
            t = psG.tile([P, 512], F32, tag="g", name="qk_ps")
            for p in range(KC + 1):
                nc.tensor.matmul(
                    t[:],
                    wqk_sb[p][:, P * h : P * (h + 1)],
                    xt_sb[p][:, 512 * n : 512 * (n + 1)],
                    start=(p == 0),
                    stop=(p == KC),
                )
            nc.vector.tensor_copy(qT[h][:, 512 * n : 512 * (n + 1)], t[0:D, :])
            nc.vector.tensor_copy(kT[h][:, 512 * n : 512 * (n + 1)], t[D : 2 * D, :])

        def v_proj(j):
            t = psG.tile([P, 512], F32, tag="g", name="v_ps")
            tv = t[:, 0 : HPC * D]
            for p in range(KC + 1):
                nc.tensor.matmul(
                    tv,
                    xt_sb[p][:, P * j : P * (j + 1)],
                    wv_sb[p][:],
                    start=(p == 0),
                    stop=(p == KC),
                )
            nc.vector.tensor_copy(
                vsb[:, j, :, 0:D], tv.rearrange("p (h d) -> p h d", d=D)
            )

        def make_epilogue(n):
            """5 thunks: 4 out-proj q-blocks into a shared staging tile + 1 DMA."""
            hold = {}

            def qb_thunk(b):
                if "ysb" not in hold:
                    hold["ysb"] = pys.tile([P, 4, C], BF16, tag="ysb", name=f"ysb{n}")
                ysb = hold["ysb"]
                qb = 4 * n + b
                ya = psG.tile([P, 512], F32, tag="g", name="ya_ps")
                yb = psG.tile([P, 256], F32, tag="g", name="yb_ps")
                csl = slice(P * qb, P * (qb + 1))
                nc.tensor.matmul(ya[:], aT2[:, csl], wo2_sb[:, 0:512], start=True, stop=False)
                nc.tensor.matmul(ya[:], aT1[0:D, csl], wo1_sb[:, 0:512], start=False, stop=True)
                nc.tensor.matmul(yb[:], aT2[:, csl], wo2_sb[:, 512:768], start=True, stop=False)
                nc.tensor.matmul(yb[:], aT1[0:D, csl], wo1_sb[:, 512:768], start=False, stop=True)
                nc.vector.tensor_copy(ysb[:, b, 0:512], ya[:])
                nc.vector.tensor_copy(ysb[:, b, 512:768], yb[:])

            def dma_thunk():
                nc.sync.dma_start(y[n].rearrange("b p c -> p b c"), hold["ysb"][:])

            return [lambda b=b: qb_thunk(b) for b in range(4)] + [dma_thunk]

        # ---- attention pair layout ----
        # (chunk j, col base in the 1024-wide pair tile, q offset in the
        # 512-wide query tile, width)
        def pair_layout(n, p):
            c0, c1 = 2 * p, 2 * p + 1
            if p < 2 * n:  # both full chunks
                return [(c0, 0, 0, 512), (c1, 512, 0, 512)], (0, 1024)
            if p == 2 * n:  # diagonal chunks v=0,1
                return [(c0, 0, 0, 512), (c1, 512, 128, 384)], (0, 896)
            # diagonal chunks v=2,3
            return [(c0, 256, 256, 256), (c1, 512, 384, 128)], (256, 640)

        def make_avfin(h, n, pts, asbp, asbh, last=False):
            """Block-major AV + normalize thunks for group (h, n): PSUM
            accumulation groups must be sequential PER BANK (verified
            empirically: interleaved same-bank groups corrupt; cross-bank
            interleaving is fine). For the last group, block 2 accumulates in
            a psS slot so blocks 2/3 can pre-run with open groups, leaving
            almost nothing after the final exp."""
            AC = psA.tile([P, 4, D + 1], F32, tag="ac", name=f"ac{h}_{n}")
            hold = {}

            def acr_for(b):
                if last and b == 2:
                    if "AC2" not in hold:
                        hold["AC2"] = psS.tile([P, D + 1], F32, tag="sp", name="ac2L")
                    return hold["AC2"][:, :]
                return AC[:, b, :]

            def avb(b, lo, hi):
                acr = acr_for(b)
                first = 2 * lo  # first chunk index in span
                for chunks, PTt in pts[lo:hi]:
                    for j, base, qoff, width in chunks:
                        b0 = qoff // P
                        if b < b0:
                            continue
                        col = base + P * b - qoff
                        nc.tensor.matmul(
                            acr,
                            PTt[:, col : col + P],
                            vsb[:, j, h, :],
                            start=(j == 0),
                            stop=(j == 4 * n + b),
                            skip_group_check=True,
                        )

            def dest_for(b):
                if h < 2:
                    return asbp[:, b, D * h : D * (h + 1)]
                return asbh[:, b, 0:D]

            def fin(bs):
                if not last:
                    r_sb = prr.tile([P, len(bs)], F32, tag="r", name="r_sb")
                    nc.vector.reciprocal(r_sb[:], AC[:, bs[0] : bs[0] + len(bs), D])
                    for i, b in enumerate(bs):
                        nc.vector.tensor_scalar_mul(
                            dest_for(b), AC[:, b, 0:D], r_sb[:, i : i + 1]
                        )
                    return
                for b in bs:
                    acr = acr_for(b)
                    r_sb = prr.tile([P, 1], F32, tag="r", name="r_sb")
                    nc.vector.reciprocal(r_sb[:], acr[:, D : D + 1])
                    nc.vector.tensor_scalar_mul(dest_for(b), acr[:, 0:D], r_sb[:])

            npr = 2 * n + 2
            spans = [(lo, min(lo + 4, npr)) for lo in range(0, npr, 4)]
            thunks = [
                (lambda b=b, lo=lo, hi=hi: avb(b, lo, hi))
                for b in range(4)
                for lo, hi in spans
            ]
            thunks.append(lambda: fin((0, 1, 2, 3)))
            return thunks, avb, fin

        # ---- global pipeline ----
        q_soft = []  # AV/fin, transposes, epilogues: drain ~ASAP, FIFO
        q_hard = []  # projections: drain EARLY (their DVE copies feed S mms)

        def pop_work(slots_left):
            while q_hard and len(q_hard) >= slots_left:
                q_hard.pop(0)()
            if len(q_soft) > 6:
                q_soft.pop(0)()
                q_soft.pop(0)()
            elif q_hard:
                q_hard.pop(0)()
            elif q_soft:
                q_soft.pop(0)()

        def transpose_thunk(dst, a, m):
            return lambda: nc.sync.dma_start_transpose(
                dst[:, 512 * m : 512 * (m + 1)].rearrange("a (b c) -> a b c", c=P),
                a[:],
            )

        def transpose_qb(dst, a, m, b):
            qb = 4 * m + b
            tr = psG.tile([P, P], BF16, tag="g", name="tr_ps")
            nc.tensor.transpose(tr[:], a[:, b, :], tri_sb[:, P : 2 * P])
            nc.vector.tensor_copy(dst[:, P * qb : P * (qb + 1)], tr[:])

        qk_proj(0, 0)
        for j in range(4):
            v_proj(j)
        q_hard.extend([lambda: qk_proj(1, 0), lambda: qk_proj(2, 0)])
        q_hard.extend([(lambda j=j: v_proj(j)) for j in range(4, 8)])

        for n in range(NI):
            asbp = pasb.tile([P, 4, P], BF16, tag="asbp", name=f"asbp{n}")
            asbh = pasb.tile([P, 4, P], BF16, tag="asbh", name=f"asbh{n}")
            nc.gpsimd.memset(asbh[:, :, D : 2 * D], 0.0)
            if 1 <= n < NI - 1:
                q_hard.extend([(lambda j=j: v_proj(j)) for j in range(4 * n + 4, 4 * n + 8)])
            if n + 1 < NI:
                q_hard.extend([(lambda h=h, m=n + 1: qk_proj(h, m)) for h in range(HPC)])
            npairs = 2 * n + 2
            slots = HPC * npairs
            last_group = n == NI - 1
            if EAGER_TAIL and last_group:
                ysbL = pys.tile([P, 4, C], BF16, tag="ysb", name="ysbL")

                def epi_qb_last(b):
                    qb = 4 * (NI - 1) + b
                    ya = psG.tile([P, 512], F32, tag="g", name="ya_ps")
                    yb = psG.tile([P, 256], F32, tag="g", name="yb_ps")
                    csl = slice(P * qb, P * (qb + 1))
                    nc.tensor.matmul(ya[:], aT2[:, csl], wo2_sb[:, 0:512], start=True, stop=False)
                    nc.tensor.matmul(ya[:], aT1[0:D, csl], wo1_sb[:, 0:512], start=False, stop=True)
                    nc.tensor.matmul(yb[:], aT2[:, csl], wo2_sb[:, 512:768], start=True, stop=False)
                    nc.tensor.matmul(yb[:], aT1[0:D, csl], wo1_sb[:, 512:768], start=False, stop=True)
                    nc.vector.tensor_copy(ysbL[:, b, 0:512], ya[:])
                    nc.vector.tensor_copy(ysbL[:, b, 512:768], yb[:])

                def dma_half_last(b0):
                    nc.sync.dma_start(
                        y[NI - 1, b0 : b0 + 2].rearrange("b p c -> p b c"),
                        ysbL[:, b0 : b0 + 2, :],
                    )

            for h in range(HPC):
                pts = []
                avfin, e_avb, e_fin = make_avfin(
                    h, n, pts, asbp, asbh, last=(EAGER_TAIL and last_group and h == 2)
                )
                eager = False
                for p in range(npairs):
                    chunks, (e0, e1) = pair_layout(n, p)
                    SP = psS.tile([P, 1024], F32, tag="sp", name="sp_ps")
                    PTt = ppt.tile([P, 1024], BF16, tag="pt", name="pt_sb")
                    for j, base, qoff, width in chunks:
                        nc.tensor.matmul(
                            SP[:, base : base + width],
                            kT[h][:, P * j : P * (j + 1)],
                            qT[h][:, 512 * n + qoff : 512 * n + qoff + width],
                            start=True,
                            stop=True,
                        )
                    nc.scalar.activation(
                        PTt[:, e0:e1], SP[:, e0:e1],
                        mybir.ActivationFunctionType.Exp, scale=0.125,
                    )
                    if p >= 2 * n:  # diagonal pair: triangle-mask leading blocks
                        for j, base, qoff, width in chunks:
                            nc.vector.tensor_mul(
                                PTt[:, base : base + P], PTt[:, base : base + P],
                                tri_sb[:, 0:P],
                            )
                    pts.append((chunks, PTt))
                    slots -= 1
                    if eager and p == 2 * n + 1:
                        # diag A complete: blocks 0,1 finished -> first tail half
                        while q_soft:
                            q_soft.pop(0)()
                        e_avb(0)
                        e_avb(1)
                        e_fin((0, 1))
                        for b in (0, 1):
                            transpose_qb(aT2, asbp, n, b)
                            transpose_qb(aT1, asbh, n, b)
                        epi_qb_last(0)
                        epi_qb_last(1)
                        dma_half_last(0)
                    elif not eager:
                        pop_work(slots + 1)
                if eager:
                    # diag B complete: blocks 2,3 + second tail half
                    e_avb(2)
                    e_avb(3)
                    e_fin((2, 3))
                    for b in (2, 3):
                        transpose_qb(aT2, asbp, n, b)
                        transpose_qb(aT1, asbh, n, b)
                    epi_qb_last(2)
                    epi_qb_last(3)
                    dma_half_last(2)
                else:
                    q_soft.extend(avfin)
            while q_hard:
                q_hard.pop(0)()
            q_soft.append(transpose_thunk(aT2, asbp, n))
            q_soft.append(transpose_thunk(aT1, asbh, n))
            if n > 0:
                q_soft.extend(make_epilogue(n - 1))
        while q_soft:
            q_soft.pop(0)()
        for th in make_epilogue(NI - 1):
            th()

        if DEBUG:
            for h in range(HPC):
                nc.sync.dma_start(dbg[f"qT{h}_d"][:], qT[h][:])
                nc.sync.dma_start(dbg[f"kT{h}_d"][:], kT[h][:])
            nc.sync.dma_start(dbg["vsb_d"][:], vsb[:])
            nc.sync.dma_start(dbg["aT2_d"][:], aT2[:])
            nc.sync.dma_start(dbg["aT1_d"][:], aT1[:])

    nc.compile()
    return nc


def _inputs_for_core(c, x, w_qkv, b_qkv, w_out, tri):
    b, g = divmod(c, 4)
    h0 = HPC * g
    xt = np.empty((C + 1, T), np.float32)
    xt[:C] = x[b].T
    xt[C] = 1.0
    wfull = np.concatenate([w_qkv, b_qkv[None, :]], axis=0)  # [C+1, 3C]
    qk_cols = []
    for h in range(h0, h0 + HPC):
        qk_cols.extend(range(D * h, D * h + D))
        qk_cols.extend(range(C + D * h, C + D * h + D))
    return {
        "xt": xt.astype(NPBF16),
        "wqk": np.ascontiguousarray(wfull[:, qk_cols]).astype(NPBF16),
        "wv": np.ascontiguousarray(
            wfull[:, 2 * C + D * h0 : 2 * C + D * (h0 + HPC)]
        ).astype(NPBF16),
        "wo2": np.ascontiguousarray(w_out[D * h0 : D * (h0 + 2), :]).astype(NPBF16),
        "wo1": np.ascontiguousarray(w_out[D * (h0 + 2) : D * (h0 + 3), :]).astype(
            NPBF16
        ),
        "tri": tri,
    }


def kernel(x, w_qkv, b_qkv, w_out, b_out):
    global _prog, LAST, _last_in_maps
    x = np.asarray(x, np.float32)
    w_qkv = np.asarray(w_qkv, np.float32)
    b_qkv = np.asarray(b_qkv, np.float32)
    w_out = np.asarray(w_out, np.float32)
    b_out = np.asarray(b_out, np.float32)
    if _prog is None:
        _prog = _build()
    tri = np.concatenate(
        [
            (np.arange(P)[None, :] >= np.arange(P)[:, None]).astype(NPBF16),
            np.eye(P, dtype=NPBF16),
        ],
        axis=1,
    )
    in_maps = [
        _inputs_for_core(c, x, w_qkv, b_qkv, w_out, tri) for c in range(NCORES)
    ]
    _last_in_maps = in_maps
    LAST = run_bass_kernel_spmd(_prog, in_maps, list(range(NCORES)))
    out = np.zeros((B, T, C), np.float32)
    for c in range(NCORES):
        out[c // 4] += np.asarray(LAST.results[c]["y"], np.float32).reshape(T, C)
    out += b_out[None, None, :]
    return out
